# revision 3
# baseline (speedup 1.0000x reference)
"""MoE MLP (top-2 of 8 experts + shared expert) Trainium2 kernel.

Strategy (8 NeuronCores, SPMD — one NEFF, per-core data):
  - Host computes the router (logits/top-2/softmax/aux-loss) in numpy, gathers
    the tokens routed to each expert, and pads to a fixed capacity C.
  - Core e runs expert e's MLP over its gathered tokens (dense, static shapes)
    with the top-2 combine weight folded into the output, PLUS the shared
    expert's MLP over a 1/8 token slice (expert-parallel + token-parallel
    shared, per the expert-parallel sharding hint).
  - All matmuls run in bf16 (f32 PSUM accumulation); activations/weights are
    pre-transposed and block-laid-out on the host so every DMA is large and
    per-partition contiguous.
  - Host scatter-adds the expert outputs and shared slices back into the full
    (B, L, D) output and returns (out, aux_loss) like the reference.

Device pipeline per 512-token chunk:
  phase 1: h_g = Wg @ x^T, h_u = Wu @ x^T (I on partitions), a = silu(h_g)*h_u
  phase 2: y^T = Wd @ a (D on partitions), y *= combine, DMA out.
"""

import os

import numpy as np
import ml_dtypes

import concourse.bacc as bacc
import concourse.mybir as mybir
import concourse.tile as tile
from concourse.bass_utils import run_bass_kernel_spmd

BF16 = mybir.dt.bfloat16
F32 = mybir.dt.float32
SILU = mybir.ActivationFunctionType.Silu
MULT = mybir.AluOpType.mult

# Problem constants (hardcoded per contract).
B, L, D, I, E, TOPK = 2, 4096, 2048, 5632, 8, 2
T = B * L                      # 8192 tokens
KD = D // 128                  # 16 contraction tiles
IT = I // 128                  # 44 intermediate tiles
NIB = IT // 4                  # 11 i-blocks of 4 tiles
DT = D // 128                  # 16 output tiles
NDG = DT // 2                  # 8 d-groups of 2 tiles
C = 2112                       # per-expert token capacity (max observed 2099)
CS = T // 8                    # shared-expert tokens per core (1024)
AUX_COEF = 0.01

bf16 = ml_dtypes.bfloat16

_CHUNKS_E = [(0, 512), (512, 512), (1024, 512), (1536, 512), (2048, 64)]
_CHUNKS_S = [(0, 512), (512, 512)]


def _emit_job(nc, pools, x_d, wg_d, wu_d, wd_d, comb_sb, y_d, chunks):
    px, pw, pwd, pa, pt, po, pp, pq = pools
    for t0, ct in chunks:
        xc = px.tile([128, KD, ct], BF16, tag="xc")
        nc.sync.dma_start(xc[:], x_d[:, :, t0:t0 + ct].rearrange("k p t -> p k t"))
        aT = []
        for ib in range(NIB):
            wg_t = pw.tile([128, KD, 512], BF16, tag="wg")
            nc.sync.dma_start(wg_t[:], wg_d[ib])
            wu_t = pw.tile([128, KD, 512], BF16, tag="wu")
            nc.sync.dma_start(wu_t[:], wu_d[ib])
            for it4 in range(4):
                ps_g = pp.tile([128, ct], F32, tag="ps1")
                ps_u = pp.tile([128, ct], F32, tag="ps1")
                sl = it4 * 128
                for k in range(KD):
                    nc.tensor.matmul(ps_g[:], wg_t[:, k, sl:sl + 128], xc[:, k, :],
                                     start=(k == 0), stop=(k == KD - 1))
                for k in range(KD):
                    nc.tensor.matmul(ps_u[:], wu_t[:, k, sl:sl + 128], xc[:, k, :],
                                     start=(k == 0), stop=(k == KD - 1))
                st = pt.tile([128, ct], F32, tag="sl")
                nc.scalar.activation(st[:], ps_g[:], SILU)
                a = pa.tile([128, ct], BF16, tag=f"aT{len(aT)}")
                nc.vector.tensor_tensor(a[:], st[:], ps_u[:], MULT)
                aT.append(a)
        for dg in range(NDG):
            psY = pq.tile([128, 2, ct], F32, tag="psY")
            for ib in range(NIB):
                wd_t = pwd.tile([128, 4, 2, 128], BF16, tag="wd")
                nc.sync.dma_start(wd_t[:], wd_d[dg, ib])
                for it4 in range(4):
                    i = ib * 4 + it4
                    for dt2 in range(2):
                        nc.tensor.matmul(psY[:, dt2, :], wd_t[:, it4, dt2, :],
                                         aT[i][:], start=(i == 0), stop=(i == IT - 1))
            ysb = po.tile([128, 2, ct], F32, tag="ysb")
            if comb_sb is None:
                nc.scalar.copy(ysb[:], psY[:])
            else:
                for dt2 in range(2):
                    nc.vector.tensor_tensor(ysb[:, dt2, :], psY[:, dt2, :],
                                            comb_sb[:, t0:t0 + ct], MULT)
            nc.sync.dma_start(
                y_d[2 * dg:2 * dg + 2, :, t0:t0 + ct].rearrange("d p t -> p d t"),
                ysb[:])


def _build():
    nc = bacc.Bacc("TRN2", target_bir_lowering=False)
    xe = nc.dram_tensor("xe", [KD, 128, C], BF16, kind="ExternalInput")
    xs = nc.dram_tensor("xs", [KD, 128, CS], BF16, kind="ExternalInput")
    wge = nc.dram_tensor("wge", [NIB, 128, KD, 512], BF16, kind="ExternalInput")
    wue = nc.dram_tensor("wue", [NIB, 128, KD, 512], BF16, kind="ExternalInput")
    wde = nc.dram_tensor("wde", [NDG, NIB, 128, 4, 2, 128], BF16, kind="ExternalInput")
    wgs = nc.dram_tensor("wgs", [NIB, 128, KD, 512], BF16, kind="ExternalInput")
    wus = nc.dram_tensor("wus", [NIB, 128, KD, 512], BF16, kind="ExternalInput")
    wds = nc.dram_tensor("wds", [NDG, NIB, 128, 4, 2, 128], BF16, kind="ExternalInput")
    comb = nc.dram_tensor("comb", [128, C], F32, kind="ExternalInput")
    ye = nc.dram_tensor("ye", [DT, 128, C], F32, kind="ExternalOutput")
    ys = nc.dram_tensor("ys", [DT, 128, CS], F32, kind="ExternalOutput")

    with tile.TileContext(nc) as tc:
        with (
            tc.tile_pool(name="px", bufs=2) as px,
            tc.tile_pool(name="pw", bufs=2) as pw,
            tc.tile_pool(name="pwd", bufs=3) as pwd,
            tc.tile_pool(name="pa", bufs=1) as pa,
            tc.tile_pool(name="pt", bufs=2) as pt,
            tc.tile_pool(name="po", bufs=2) as po,
            tc.tile_pool(name="pc", bufs=1) as pc,
            tc.tile_pool(name="pp", bufs=4, space="PSUM") as pp,
            tc.tile_pool(name="pq", bufs=2, space="PSUM") as pq,
        ):
            comb_sb = pc.tile([128, C], F32, tag="comb")
            nc.sync.dma_start(comb_sb[:], comb[:])
            pools = (px, pw, pwd, pa, pt, po, pp, pq)
            _emit_job(nc, pools, xe, wge, wue, wde, comb_sb, ye, _CHUNKS_E)
            _emit_job(nc, pools, xs, wgs, wus, wds, None, ys, _CHUNKS_S)
    nc.compile()
    return nc


def _block_gu(w):
    # w: (I, D) f32 -> (NIB, 128, KD, 512) bf16, [ib, p, k, ii] with d=k*128+p,
    # i=ib*512+ii (i.e. blocked layout of w.T, contraction-major).
    return np.ascontiguousarray(
        w.T.reshape(KD, 128, NIB, 512).transpose(2, 1, 0, 3)).astype(bf16)


def _block_d(w):
    # w: (D, I) f32 -> (NDG, NIB, 128, 4, 2, 128) bf16 from w.T (I, D):
    # i = (ib*4+it4)*128 + p, d = (dg*2+dt2)*128 + f.
    return np.ascontiguousarray(
        w.T.reshape(NIB, 4, 128, NDG, 2, 128).transpose(3, 0, 2, 1, 4, 5)).astype(bf16)


def _route(x, gate_w):
    # Replicates jax.lax.top_k(logits, 2) + softmax in numpy.
    logits = x @ gate_w.T                       # (T, E) f32
    ar = np.arange(T)
    i1 = logits.argmax(1)
    v1 = logits[ar, i1]
    l2 = logits.copy()
    l2[ar, i1] = -np.inf
    i2 = l2.argmax(1)
    v2 = logits[ar, i2]
    # softmax over (v1, v2), v1 >= v2
    e2 = np.exp((v2 - v1).astype(np.float64))
    p1 = (1.0 / (1.0 + e2)).astype(np.float32)
    p2 = 1.0 - p1
    # aux loss (matches reference formula)
    m = logits.max(1, keepdims=True)
    p = np.exp((logits - m).astype(np.float64))
    p /= p.sum(1, keepdims=True)
    mean_prob = p.mean(0)
    counts = np.bincount(i1, minlength=E) + np.bincount(i2, minlength=E)
    fraction = counts / float(T)
    aux = np.float32(AUX_COEF * float((fraction * mean_prob).sum()) * E)
    return logits, i1, i2, p1, p2, aux


def _mlp_host(xr, wg, wu, wd):
    g = xr @ wg.T
    u = xr @ wu.T
    return (g / (1.0 + np.exp(-g)) * u) @ wd.T


def prepare(hidden_states, gate_w, expert_gate, expert_up, expert_down,
            shared_gate, shared_up, shared_down):
    """Host-side routing + sharding. Returns (in_maps, meta)."""
    x = np.ascontiguousarray(np.asarray(hidden_states, np.float32).reshape(T, D))
    gate_w = np.asarray(gate_w, np.float32)
    expert_gate = np.asarray(expert_gate, np.float32)
    expert_up = np.asarray(expert_up, np.float32)
    expert_down = np.asarray(expert_down, np.float32)
    shared_gate = np.asarray(shared_gate, np.float32)
    shared_up = np.asarray(shared_up, np.float32)
    shared_down = np.asarray(shared_down, np.float32)

    logits, i1, i2, p1, p2, aux = _route(x, gate_w)

    xT = np.ascontiguousarray(x.T).astype(bf16)       # (D, T) bf16
    wgs_b = _block_gu(shared_gate)
    wus_b = _block_gu(shared_up)
    wds_b = _block_d(shared_down)

    in_maps = []
    idx_list = []
    overflow = []
    for e in range(E):
        idx = np.nonzero((i1 == e) | (i2 == e))[0]
        cnt = len(idx)
        if cnt > C:
            overflow.append((e, idx[C:]))
            idx = idx[:C]
            cnt = C
        idx_list.append(idx)
        xg = np.zeros((KD, 128, C), bf16)
        xg[:, :, :cnt] = xT[:, idx].reshape(KD, 128, cnt)
        cvec = np.where(i1[idx] == e, p1[idx], p2[idx]).astype(np.float32)
        comb = np.zeros((128, C), np.float32)
        comb[:, :cnt] = cvec[None, :]
        in_maps.append({
            "xe": xg,
            "xs": np.ascontiguousarray(
                xT[:, e * CS:(e + 1) * CS]).reshape(KD, 128, CS),
            "wge": _block_gu(expert_gate[e]),
            "wue": _block_gu(expert_up[e]),
            "wde": _block_d(expert_down[e]),
            "wgs": wgs_b,
            "wus": wus_b,
            "wds": wds_b,
            "comb": comb,
        })

    meta = dict(idx_list=idx_list, overflow=overflow, i1=i1, i2=i2, p1=p1, p2=p2,
                aux=aux, x=x, expert_gate=expert_gate, expert_up=expert_up,
                expert_down=expert_down)
    return in_maps, meta


def assemble(results, meta):
    out = np.empty((T, D), np.float32)
    for c in range(E):
        out[c * CS:(c + 1) * CS] = results[c]["ys"].reshape(D, CS).T
    for e in range(E):
        idx = meta["idx_list"][e]
        out[idx] += results[e]["ye"].reshape(D, C)[:, :len(idx)].T
    i1, p1, p2 = meta["i1"], meta["p1"], meta["p2"]
    for e, idx in meta["overflow"]:
        cvec = np.where(i1[idx] == e, p1[idx], p2[idx]).astype(np.float32)
        out[idx] += _mlp_host(meta["x"][idx], meta["expert_gate"][e],
                              meta["expert_up"][e],
                              meta["expert_down"][e]) * cvec[:, None]
    return out.reshape(B, L, D), meta["aux"]


def kernel(hidden_states, gate_w, expert_gate, expert_up, expert_down,
           shared_gate, shared_up, shared_down):
    in_maps, meta = prepare(hidden_states, gate_w, expert_gate, expert_up,
                            expert_down, shared_gate, shared_up, shared_down)
    nc = _build()
    res = run_bass_kernel_spmd(nc, in_maps, core_ids=list(range(8)))
    kernel.last_results = res
    return assemble(res.results, meta)


# revision 7
# speedup vs baseline: 70.6291x; 70.6291x over previous
"""MoE MLP (top-2 of 8 experts + shared expert) Trainium2 kernel.

Strategy (8 NeuronCores, SPMD — one NEFF, per-core data):
  - Host computes the router (logits/top-2/softmax/aux-loss) in numpy, gathers
    the tokens routed to each expert, and pads to a fixed capacity C.
  - Core e runs expert e's MLP over its gathered tokens (dense, static shapes)
    with the top-2 combine weight folded into the output, PLUS the shared
    expert's MLP over a 1/8 token slice (expert-parallel + token-parallel
    shared, per the expert-parallel sharding hint).
  - All matmuls run in bf16 (f32 PSUM accumulation); activations/weights are
    pre-transposed and block-laid-out on the host so every DMA is large and
    per-partition contiguous.
  - Host scatter-adds the expert outputs and shared slices back into the full
    (B, L, D) output and returns (out, aux_loss) like the reference.

Device pipeline per 512-token chunk:
  phase 1: h_g = Wg @ x^T, h_u = Wu @ x^T (I on partitions), a = silu(h_g)*h_u
  phase 2: y^T = Wd @ a (D on partitions), y *= combine, DMA out.
"""

import numpy as np
import ml_dtypes

import concourse.bacc as bacc
import concourse.mybir as mybir
import concourse.tile as tile
from concourse.bass_utils import run_bass_kernel_spmd

BF16 = mybir.dt.bfloat16
F32 = mybir.dt.float32
SILU = mybir.ActivationFunctionType.Silu
MULT = mybir.AluOpType.mult

# Problem constants (hardcoded per contract).
B, L, D, I, E, TOPK = 2, 4096, 2048, 5632, 8, 2
T = B * L                      # 8192 tokens
KD = D // 128                  # 16 contraction tiles
IT = I // 128                  # 44 intermediate tiles
NIB = IT // 4                  # 11 i-blocks of 4 tiles
DT = D // 128                  # 16 output tiles
NDG = DT // 2                  # 8 d-groups of 2 tiles
C = 2048                       # per-expert token capacity; overflow runs on host
CS = T // 8                    # shared-expert tokens per core (1024)
AUX_COEF = 0.01

bf16 = ml_dtypes.bfloat16

_CHUNKS_E = [(0, 512), (512, 512), (1024, 512), (1536, 512)]
_CHUNKS_S = [(0, 512), (512, 512)]


def _emit_job(nc, pools, x_d, wg_d, wu_d, wd_d, comb_sb, y_d, chunks):
    px, pw, pwd, pa, pt, po, pp, pq = pools
    for t0, ct in chunks:
        xc = px.tile([128, KD, ct], BF16, tag="xc")
        nc.sync.dma_start(xc[:], x_d[:, :, t0:t0 + ct].rearrange("k p t -> p k t"))
        aT = []
        for ib in range(NIB):
            wg_t = pw.tile([128, KD, 512], BF16, tag="wg")
            nc.sync.dma_start(wg_t[:], wg_d[ib])
            wu_t = pw.tile([128, KD, 512], BF16, tag="wu")
            nc.sync.dma_start(wu_t[:], wu_d[ib])
            for it4 in range(4):
                ps_g = pp.tile([128, ct], F32, tag="ps1")
                ps_u = pp.tile([128, ct], F32, tag="ps1")
                sl = it4 * 128
                for k in range(KD):
                    nc.tensor.matmul(ps_g[:], wg_t[:, k, sl:sl + 128], xc[:, k, :],
                                     start=(k == 0), stop=(k == KD - 1))
                for k in range(KD):
                    nc.tensor.matmul(ps_u[:], wu_t[:, k, sl:sl + 128], xc[:, k, :],
                                     start=(k == 0), stop=(k == KD - 1))
                st = pt.tile([128, ct], F32, tag="sl")
                nc.scalar.activation(st[:], ps_g[:], SILU)
                a = pa.tile([128, ct], BF16, tag=f"aT{len(aT)}")
                nc.vector.tensor_tensor(a[:], st[:], ps_u[:], MULT)
                aT.append(a)
        for dg in range(NDG):
            psY = pq.tile([128, 2, ct], F32, tag="psY")
            for ib in range(NIB):
                wd_t = pwd.tile([128, 4, 2, 128], BF16, tag="wd")
                nc.sync.dma_start(wd_t[:], wd_d[dg, ib])
                for it4 in range(4):
                    i = ib * 4 + it4
                    for dt2 in range(2):
                        nc.tensor.matmul(psY[:, dt2, :], wd_t[:, it4, dt2, :],
                                         aT[i][:], start=(i == 0), stop=(i == IT - 1))
            ysb = po.tile([128, 2, ct], F32, tag="ysb")
            if comb_sb is None:
                nc.scalar.copy(ysb[:], psY[:])
            else:
                for dt2 in range(2):
                    nc.vector.tensor_tensor(ysb[:, dt2, :], psY[:, dt2, :],
                                            comb_sb[:, t0:t0 + ct], MULT)
            nc.sync.dma_start(
                y_d[2 * dg:2 * dg + 2, :, t0:t0 + ct].rearrange("d p t -> p d t"),
                ysb[:])


_NC_CACHE = None


def _build():
    global _NC_CACHE
    if _NC_CACHE is not None:
        return _NC_CACHE
    nc = bacc.Bacc("TRN2", target_bir_lowering=False)
    xe = nc.dram_tensor("xe", [KD, 128, C], BF16, kind="ExternalInput")
    xs = nc.dram_tensor("xs", [KD, 128, CS], BF16, kind="ExternalInput")
    wge = nc.dram_tensor("wge", [NIB, 128, KD, 512], BF16, kind="ExternalInput")
    wue = nc.dram_tensor("wue", [NIB, 128, KD, 512], BF16, kind="ExternalInput")
    wde = nc.dram_tensor("wde", [NDG, NIB, 128, 4, 2, 128], BF16, kind="ExternalInput")
    wgs = nc.dram_tensor("wgs", [NIB, 128, KD, 512], BF16, kind="ExternalInput")
    wus = nc.dram_tensor("wus", [NIB, 128, KD, 512], BF16, kind="ExternalInput")
    wds = nc.dram_tensor("wds", [NDG, NIB, 128, 4, 2, 128], BF16, kind="ExternalInput")
    comb = nc.dram_tensor("comb", [128, C], F32, kind="ExternalInput")
    ye = nc.dram_tensor("ye", [DT, 128, C], F32, kind="ExternalOutput")
    ys = nc.dram_tensor("ys", [DT, 128, CS], F32, kind="ExternalOutput")

    with tile.TileContext(nc) as tc:
        with (
            tc.tile_pool(name="px", bufs=2) as px,
            tc.tile_pool(name="pw", bufs=2) as pw,
            tc.tile_pool(name="pwd", bufs=3) as pwd,
            tc.tile_pool(name="pa", bufs=1) as pa,
            tc.tile_pool(name="pt", bufs=2) as pt,
            tc.tile_pool(name="po", bufs=2) as po,
            tc.tile_pool(name="pc", bufs=1) as pc,
            tc.tile_pool(name="pp", bufs=4, space="PSUM") as pp,
            tc.tile_pool(name="pq", bufs=2, space="PSUM") as pq,
        ):
            comb_sb = pc.tile([128, C], F32, tag="comb")
            nc.sync.dma_start(comb_sb[:], comb[:])
            pools = (px, pw, pwd, pa, pt, po, pp, pq)
            _emit_job(nc, pools, xe, wge, wue, wde, comb_sb, ye, _CHUNKS_E)
            _emit_job(nc, pools, xs, wgs, wus, wds, None, ys, _CHUNKS_S)
    nc.compile()
    _NC_CACHE = nc
    return nc


def _block_gu(w):
    # w: (I, D) f32 -> (NIB, 128, KD, 512) bf16, [ib, p, k, ii] with d=k*128+p,
    # i=ib*512+ii (i.e. blocked layout of w.T, contraction-major).
    return np.ascontiguousarray(
        w.T.reshape(KD, 128, NIB, 512).transpose(2, 1, 0, 3)).astype(bf16)


def _block_d(w):
    # w: (D, I) f32 -> (NDG, NIB, 128, 4, 2, 128) bf16 from w.T (I, D):
    # i = (ib*4+it4)*128 + p, d = (dg*2+dt2)*128 + f.
    return np.ascontiguousarray(
        w.T.reshape(NIB, 4, 128, NDG, 2, 128).transpose(3, 0, 2, 1, 4, 5)).astype(bf16)


def _route(x, gate_w):
    # Replicates jax.lax.top_k(logits, 2) + softmax in numpy.
    logits = x @ gate_w.T                       # (T, E) f32
    ar = np.arange(T)
    i1 = logits.argmax(1)
    v1 = logits[ar, i1]
    l2 = logits.copy()
    l2[ar, i1] = -np.inf
    i2 = l2.argmax(1)
    v2 = logits[ar, i2]
    # softmax over (v1, v2), v1 >= v2
    e2 = np.exp((v2 - v1).astype(np.float64))
    p1 = (1.0 / (1.0 + e2)).astype(np.float32)
    p2 = 1.0 - p1
    # aux loss (matches reference formula)
    m = logits.max(1, keepdims=True)
    p = np.exp((logits - m).astype(np.float64))
    p /= p.sum(1, keepdims=True)
    mean_prob = p.mean(0)
    counts = np.bincount(i1, minlength=E) + np.bincount(i2, minlength=E)
    fraction = counts / float(T)
    aux = np.float32(AUX_COEF * float((fraction * mean_prob).sum()) * E)
    return logits, i1, i2, p1, p2, aux


def _mlp_host(xr, wg, wu, wd):
    g = xr @ wg.T
    u = xr @ wu.T
    return (g / (1.0 + np.exp(-g)) * u) @ wd.T


def prepare(hidden_states, gate_w, expert_gate, expert_up, expert_down,
            shared_gate, shared_up, shared_down):
    """Host-side routing + sharding. Returns (in_maps, meta)."""
    x = np.ascontiguousarray(np.asarray(hidden_states, np.float32).reshape(T, D))
    gate_w = np.asarray(gate_w, np.float32)
    expert_gate = np.asarray(expert_gate, np.float32)
    expert_up = np.asarray(expert_up, np.float32)
    expert_down = np.asarray(expert_down, np.float32)
    shared_gate = np.asarray(shared_gate, np.float32)
    shared_up = np.asarray(shared_up, np.float32)
    shared_down = np.asarray(shared_down, np.float32)

    logits, i1, i2, p1, p2, aux = _route(x, gate_w)

    xT = np.ascontiguousarray(x.T).astype(bf16)       # (D, T) bf16
    wgs_b = _block_gu(shared_gate)
    wus_b = _block_gu(shared_up)
    wds_b = _block_d(shared_down)

    in_maps = []
    idx_list = []
    overflow = []
    for e in range(E):
        idx = np.nonzero((i1 == e) | (i2 == e))[0]
        cnt = len(idx)
        if cnt > C:
            overflow.append((e, idx[C:]))
            idx = idx[:C]
            cnt = C
        idx_list.append(idx)
        xg = np.zeros((KD, 128, C), bf16)
        xg[:, :, :cnt] = xT[:, idx].reshape(KD, 128, cnt)
        cvec = np.where(i1[idx] == e, p1[idx], p2[idx]).astype(np.float32)
        comb = np.zeros((128, C), np.float32)
        comb[:, :cnt] = cvec[None, :]
        in_maps.append({
            "xe": xg,
            "xs": np.ascontiguousarray(
                xT[:, e * CS:(e + 1) * CS]).reshape(KD, 128, CS),
            "wge": _block_gu(expert_gate[e]),
            "wue": _block_gu(expert_up[e]),
            "wde": _block_d(expert_down[e]),
            "wgs": wgs_b,
            "wus": wus_b,
            "wds": wds_b,
            "comb": comb,
        })

    meta = dict(idx_list=idx_list, overflow=overflow, i1=i1, i2=i2, p1=p1, p2=p2,
                aux=aux, x=x, expert_gate=expert_gate, expert_up=expert_up,
                expert_down=expert_down)
    return in_maps, meta


def assemble(results, meta):
    out = np.empty((T, D), np.float32)
    for c in range(E):
        out[c * CS:(c + 1) * CS] = results[c]["ys"].reshape(D, CS).T
    for e in range(E):
        idx = meta["idx_list"][e]
        out[idx] += results[e]["ye"].reshape(D, C)[:, :len(idx)].T
    i1, p1, p2 = meta["i1"], meta["p1"], meta["p2"]
    for e, idx in meta["overflow"]:
        cvec = np.where(i1[idx] == e, p1[idx], p2[idx]).astype(np.float32)
        out[idx] += _mlp_host(meta["x"][idx], meta["expert_gate"][e],
                              meta["expert_up"][e],
                              meta["expert_down"][e]) * cvec[:, None]
    return out.reshape(B, L, D), meta["aux"]


def kernel(hidden_states, gate_w, expert_gate, expert_up, expert_down,
           shared_gate, shared_up, shared_down):
    in_maps, meta = prepare(hidden_states, gate_w, expert_gate, expert_up,
                            expert_down, shared_gate, shared_up, shared_down)
    nc = _build()
    res = run_bass_kernel_spmd(nc, in_maps, core_ids=list(range(8)))
    kernel.last_results = res
    return assemble(res.results, meta)


# revision 8
# speedup vs baseline: 72.9663x; 1.0331x over previous
"""MoE MLP (top-2 of 8 experts + shared expert) Trainium2 kernel.

Strategy (8 NeuronCores, SPMD — one NEFF, per-core data):
  - Host computes the router (logits/top-2/softmax/aux-loss) in numpy, gathers
    the tokens routed to each expert, and pads to a fixed capacity C.
  - Core e runs expert e's MLP over its gathered tokens (dense, static shapes)
    with the top-2 combine weight folded into the output, PLUS the shared
    expert's MLP over a 1/8 token slice (expert-parallel + token-parallel
    shared, per the expert-parallel sharding hint).
  - All matmuls run in bf16 (f32 PSUM accumulation); activations/weights are
    pre-transposed and block-laid-out on the host so every DMA is large and
    per-partition contiguous.
  - Host scatter-adds the expert outputs and shared slices back into the full
    (B, L, D) output and returns (out, aux_loss) like the reference.

Device pipeline per 512-token chunk:
  phase 1: h_g = Wg @ x^T, h_u = Wu @ x^T (I on partitions), a = silu(h_g)*h_u
  phase 2: y^T = Wd @ a (D on partitions), y *= combine, DMA out.
"""

import numpy as np
import ml_dtypes

import concourse.bacc as bacc
import concourse.mybir as mybir
import concourse.tile as tile
from concourse.bass_utils import run_bass_kernel_spmd

BF16 = mybir.dt.bfloat16
F32 = mybir.dt.float32
SILU = mybir.ActivationFunctionType.Silu
MULT = mybir.AluOpType.mult

# Problem constants (hardcoded per contract).
B, L, D, I, E, TOPK = 2, 4096, 2048, 5632, 8, 2
T = B * L                      # 8192 tokens
KD = D // 128                  # 16 contraction tiles
IT = I // 128                  # 44 intermediate tiles
NIB = IT // 4                  # 11 i-blocks of 4 tiles
DT = D // 128                  # 16 output tiles
NDG = DT // 2                  # 8 d-groups of 2 tiles
C = 2048                       # per-expert token capacity; overflow runs on host
CS = T // 8                    # shared-expert tokens per core (1024)
AUX_COEF = 0.01

bf16 = ml_dtypes.bfloat16

_CHUNKS_E = [(0, 512), (512, 512), (1024, 512), (1536, 512)]
_CHUNKS_S = [(0, 512), (512, 512)]


def _emit_job(nc, pools, x_d, wg_d, wu_d, wd_d, comb_sb, y_d, chunks):
    px, pw, pwd, pa, pt, po, pp, pq = pools
    for t0, ct in chunks:
        xc = px.tile([128, KD, ct], BF16, tag="xc")
        nc.sync.dma_start(xc[:], x_d[:, :, t0:t0 + ct].rearrange("k p t -> p k t"))
        aT = []
        for ib in range(NIB):
            wg_t = pw.tile([128, KD, 512], BF16, tag="wg")
            nc.sync.dma_start(wg_t[:], wg_d[ib])
            wu_t = pw.tile([128, KD, 512], BF16, tag="wu")
            nc.sync.dma_start(wu_t[:], wu_d[ib])
            for it4 in range(4):
                ps_g = pp.tile([128, ct], F32, tag="ps1")
                ps_u = pp.tile([128, ct], F32, tag="ps1")
                sl = it4 * 128
                for k in range(KD):
                    nc.tensor.matmul(ps_g[:], wg_t[:, k, sl:sl + 128], xc[:, k, :],
                                     start=(k == 0), stop=(k == KD - 1))
                for k in range(KD):
                    nc.tensor.matmul(ps_u[:], wu_t[:, k, sl:sl + 128], xc[:, k, :],
                                     start=(k == 0), stop=(k == KD - 1))
                st = pt.tile([128, ct], F32, tag="sl")
                nc.scalar.activation(st[:], ps_g[:], SILU)
                a = pa.tile([128, ct], BF16, tag=f"aT{len(aT)}")
                nc.vector.tensor_tensor(a[:], st[:], ps_u[:], MULT)
                aT.append(a)
        for dg in range(NDG):
            psY = pq.tile([128, 2, ct], F32, tag="psY")
            for ib in range(NIB):
                wd_t = pwd.tile([128, 4, 2, 128], BF16, tag="wd")
                nc.sync.dma_start(wd_t[:], wd_d[dg, ib])
                for it4 in range(4):
                    i = ib * 4 + it4
                    for dt2 in range(2):
                        nc.tensor.matmul(psY[:, dt2, :], wd_t[:, it4, dt2, :],
                                         aT[i][:], start=(i == 0), stop=(i == IT - 1))
            ysb = po.tile([128, 2, ct], F32, tag="ysb")
            if comb_sb is None:
                nc.scalar.copy(ysb[:], psY[:])
            else:
                for dt2 in range(2):
                    nc.vector.tensor_tensor(ysb[:, dt2, :], psY[:, dt2, :],
                                            comb_sb[:, t0:t0 + ct], MULT)
            nc.sync.dma_start(
                y_d[2 * dg:2 * dg + 2, :, t0:t0 + ct].rearrange("d p t -> p d t"),
                ysb[:])


_NC_CACHE = None


def _build(passes=1):
    global _NC_CACHE
    if passes == 1 and _NC_CACHE is not None:
        return _NC_CACHE
    nc = bacc.Bacc("TRN2", target_bir_lowering=False)
    xe = nc.dram_tensor("xe", [KD, 128, C], BF16, kind="ExternalInput")
    xs = nc.dram_tensor("xs", [KD, 128, CS], BF16, kind="ExternalInput")
    wge = nc.dram_tensor("wge", [NIB, 128, KD, 512], BF16, kind="ExternalInput")
    wue = nc.dram_tensor("wue", [NIB, 128, KD, 512], BF16, kind="ExternalInput")
    wde = nc.dram_tensor("wde", [NDG, NIB, 128, 4, 2, 128], BF16, kind="ExternalInput")
    wgs = nc.dram_tensor("wgs", [NIB, 128, KD, 512], BF16, kind="ExternalInput")
    wus = nc.dram_tensor("wus", [NIB, 128, KD, 512], BF16, kind="ExternalInput")
    wds = nc.dram_tensor("wds", [NDG, NIB, 128, 4, 2, 128], BF16, kind="ExternalInput")
    comb = nc.dram_tensor("comb", [128, C], F32, kind="ExternalInput")
    ye = nc.dram_tensor("ye", [DT, 128, C], F32, kind="ExternalOutput")
    ys = nc.dram_tensor("ys", [DT, 128, CS], F32, kind="ExternalOutput")

    with tile.TileContext(nc) as tc:
        with (
            tc.tile_pool(name="px", bufs=2) as px,
            tc.tile_pool(name="pw", bufs=2) as pw,
            tc.tile_pool(name="pwd", bufs=3) as pwd,
            tc.tile_pool(name="pa", bufs=1) as pa,
            tc.tile_pool(name="pt", bufs=2) as pt,
            tc.tile_pool(name="po", bufs=2) as po,
            tc.tile_pool(name="pc", bufs=1) as pc,
            tc.tile_pool(name="pp", bufs=4, space="PSUM") as pp,
            tc.tile_pool(name="pq", bufs=2, space="PSUM") as pq,
        ):
            comb_sb = pc.tile([128, C], F32, tag="comb")
            nc.sync.dma_start(comb_sb[:], comb[:])
            pools = (px, pw, pwd, pa, pt, po, pp, pq)
            for _ in range(passes):
                _emit_job(nc, pools, xe, wge, wue, wde, comb_sb, ye, _CHUNKS_E)
                _emit_job(nc, pools, xs, wgs, wus, wds, None, ys, _CHUNKS_S)
    nc.compile()
    if passes == 1:
        _NC_CACHE = nc
    return nc


def _block_gu(w):
    # w: (I, D) f32 -> (NIB, 128, KD, 512) bf16, [ib, p, k, ii] with d=k*128+p,
    # i=ib*512+ii (i.e. blocked layout of w.T, contraction-major).
    return np.ascontiguousarray(
        w.T.reshape(KD, 128, NIB, 512).transpose(2, 1, 0, 3)).astype(bf16)


def _block_d(w):
    # w: (D, I) f32 -> (NDG, NIB, 128, 4, 2, 128) bf16 from w.T (I, D):
    # i = (ib*4+it4)*128 + p, d = (dg*2+dt2)*128 + f.
    return np.ascontiguousarray(
        w.T.reshape(NIB, 4, 128, NDG, 2, 128).transpose(3, 0, 2, 1, 4, 5)).astype(bf16)


def _route(x, gate_w):
    # Replicates jax.lax.top_k(logits, 2) + softmax in numpy.
    logits = x @ gate_w.T                       # (T, E) f32
    ar = np.arange(T)
    i1 = logits.argmax(1)
    v1 = logits[ar, i1]
    l2 = logits.copy()
    l2[ar, i1] = -np.inf
    i2 = l2.argmax(1)
    v2 = logits[ar, i2]
    # softmax over (v1, v2), v1 >= v2
    e2 = np.exp((v2 - v1).astype(np.float64))
    p1 = (1.0 / (1.0 + e2)).astype(np.float32)
    p2 = 1.0 - p1
    # aux loss (matches reference formula)
    m = logits.max(1, keepdims=True)
    p = np.exp((logits - m).astype(np.float64))
    p /= p.sum(1, keepdims=True)
    mean_prob = p.mean(0)
    counts = np.bincount(i1, minlength=E) + np.bincount(i2, minlength=E)
    fraction = counts / float(T)
    aux = np.float32(AUX_COEF * float((fraction * mean_prob).sum()) * E)
    return logits, i1, i2, p1, p2, aux


def _mlp_host(xr, wg, wu, wd):
    g = xr @ wg.T
    u = xr @ wu.T
    return (g / (1.0 + np.exp(-g)) * u) @ wd.T


def prepare(hidden_states, gate_w, expert_gate, expert_up, expert_down,
            shared_gate, shared_up, shared_down):
    """Host-side routing + sharding. Returns (in_maps, meta)."""
    x = np.ascontiguousarray(np.asarray(hidden_states, np.float32).reshape(T, D))
    gate_w = np.asarray(gate_w, np.float32)
    expert_gate = np.asarray(expert_gate, np.float32)
    expert_up = np.asarray(expert_up, np.float32)
    expert_down = np.asarray(expert_down, np.float32)
    shared_gate = np.asarray(shared_gate, np.float32)
    shared_up = np.asarray(shared_up, np.float32)
    shared_down = np.asarray(shared_down, np.float32)

    logits, i1, i2, p1, p2, aux = _route(x, gate_w)

    xT = np.ascontiguousarray(x.T).astype(bf16)       # (D, T) bf16
    wgs_b = _block_gu(shared_gate)
    wus_b = _block_gu(shared_up)
    wds_b = _block_d(shared_down)

    in_maps = []
    idx_list = []
    overflow = []
    for e in range(E):
        idx = np.nonzero((i1 == e) | (i2 == e))[0]
        cnt = len(idx)
        if cnt > C:
            overflow.append((e, idx[C:]))
            idx = idx[:C]
            cnt = C
        idx_list.append(idx)
        xg = np.zeros((KD, 128, C), bf16)
        xg[:, :, :cnt] = xT[:, idx].reshape(KD, 128, cnt)
        cvec = np.where(i1[idx] == e, p1[idx], p2[idx]).astype(np.float32)
        comb = np.zeros((128, C), np.float32)
        comb[:, :cnt] = cvec[None, :]
        in_maps.append({
            "xe": xg,
            "xs": np.ascontiguousarray(
                xT[:, e * CS:(e + 1) * CS]).reshape(KD, 128, CS),
            "wge": _block_gu(expert_gate[e]),
            "wue": _block_gu(expert_up[e]),
            "wde": _block_d(expert_down[e]),
            "wgs": wgs_b,
            "wus": wus_b,
            "wds": wds_b,
            "comb": comb,
        })

    meta = dict(idx_list=idx_list, overflow=overflow, i1=i1, i2=i2, p1=p1, p2=p2,
                aux=aux, x=x, expert_gate=expert_gate, expert_up=expert_up,
                expert_down=expert_down)
    return in_maps, meta


def assemble(results, meta):
    out = np.empty((T, D), np.float32)
    for c in range(E):
        out[c * CS:(c + 1) * CS] = results[c]["ys"].reshape(D, CS).T
    for e in range(E):
        idx = meta["idx_list"][e]
        out[idx] += results[e]["ye"].reshape(D, C)[:, :len(idx)].T
    i1, p1, p2 = meta["i1"], meta["p1"], meta["p2"]
    for e, idx in meta["overflow"]:
        cvec = np.where(i1[idx] == e, p1[idx], p2[idx]).astype(np.float32)
        out[idx] += _mlp_host(meta["x"][idx], meta["expert_gate"][e],
                              meta["expert_up"][e],
                              meta["expert_down"][e]) * cvec[:, None]
    return out.reshape(B, L, D), meta["aux"]


def kernel(hidden_states, gate_w, expert_gate, expert_up, expert_down,
           shared_gate, shared_up, shared_down):
    in_maps, meta = prepare(hidden_states, gate_w, expert_gate, expert_up,
                            expert_down, shared_gate, shared_up, shared_down)
    nc = _build()
    res = run_bass_kernel_spmd(nc, in_maps, core_ids=list(range(8)))
    kernel.last_results = res
    return assemble(res.results, meta)
